# revision 55
# baseline (speedup 1.0000x reference)
"""Trainium2 Bass kernel for nn_Attention (dense transformer attention block).

Full causal attention: QKV projection + RoPE + softmax(QK^T/sqrt(d) + mask)V + WO,
bsz=1, seqlen=2048, dim=4096, 32 heads x head_dim 128, fp32 I/O.

Sharding: tensor-parallel across heads on 8 NeuronCores. Core c owns heads
4c..4c+3 (wq/wk/wv output columns, attention) and wo output columns
512c..512c+512 (after an AllGather of the per-core attn^T shard along the
head axis). Host concatenates the 8 output column shards.

v3 layout: section 1 is pipelined by TOKEN QUARTERS (512 tokens). For each
quarter: Q/K projection for all 4 local heads (8 PSUM banks exactly), RoPE
(half-swap via partition-moving DMAs, not PE matmuls), V projection, and —
in causal mode — the attention for that query range immediately after
(causal attention for quarter r only needs K/V up to the diagonal). The
AllGather of quarter r's attention output therefore fires ~3 quarters
before WO consumes it, hiding the collective stack's cold-start cost and
its run-to-run variance entirely behind compute. Rotated Q stays in SBUF
(no DRAM spill round-trip). WO runs after the last attention and is paced
purely by the PE. All matmuls bf16.
"""

import ml_dtypes
import numpy as np

import concourse.bacc as bacc
import concourse.mybir as mybir
import concourse.tile as tile
from concourse.bass_utils import run_bass_kernel_spmd

# Problem constants (hardcoded per contract)
N_CORES = 8
S = 2048              # sequence length
D = 4096              # model dim
HD = 128              # head dim
NH_LOC = 4            # heads per core
DSH = 512             # per-core shard width (NH_LOC * HD)
KT = D // 128         # 32 contraction tiles over model dim
QTILES = S // 128     # 16 token tiles
QRANGES = S // 512    # 4 query ranges of 512
SCALE = float(1.0 / np.sqrt(HD))

F32 = mybir.dt.float32
BF16 = mybir.dt.bfloat16

_PROGRAMS = {}


def _build_program(mode):
    """mode: 'causal' (triu -1e9 mask), 'nomask' (zero mask), 'general'
    (arbitrary additive mask streamed from DRAM)."""
    causal = mode == "causal"
    general = mode == "general"

    nc = bacc.Bacc("TRN2", target_bir_lowering=False, debug=False,
                   num_devices=N_CORES)

    # ---- external inputs (per core) ----
    xT_d = nc.dram_tensor("xT", [D, S], BF16, kind="ExternalInput")
    wq_d = nc.dram_tensor("wq", [NH_LOC, 128, KT, HD], BF16, kind="ExternalInput")
    wk_d = nc.dram_tensor("wk", [NH_LOC, 128, KT, HD], BF16, kind="ExternalInput")
    wv_d = nc.dram_tensor("wv", [128, KT, DSH], BF16, kind="ExternalInput")
    wo_d = nc.dram_tensor("wo", [128, KT, DSH], BF16, kind="ExternalInput")
    fr_d = nc.dram_tensor("fr128", [128, S], BF16, kind="ExternalInput")
    fis_d = nc.dram_tensor("fis128", [128, S], BF16, kind="ExternalInput")
    onesmat_d = nc.dram_tensor("onesmat", [128, 128], BF16, kind="ExternalInput")
    if causal:
        # multiplicative causal mask tile in [k, q] layout (1 iff k<=q),
        # applied on the exp'd diagonal block via a DVE multiply
        trimask_d = nc.dram_tensor("trimask", [128, 128], BF16,
                                   kind="ExternalInput")
    if general:
        masktf_d = nc.dram_tensor("masktf", [S, S], F32, kind="ExternalInput")
    out_d = nc.dram_tensor("out", [S, DSH], F32, kind="ExternalOutput")

    with tile.TileContext(nc) as tc:
        with (
            tc.tile_pool(name="consts", bufs=1) as cns,
            tc.tile_pool(name="dram", bufs=1, space="DRAM") as dram,
            tc.tile_pool(name="akv", bufs=1) as akv,
            tc.tile_pool(name="ps", bufs=1, space="PSUM") as ps,
        ):
            agi = [dram.tile([DSH, 512], BF16, name=f"agi{r}") for r in range(4)]
            ago = [dram.tile([D, 512], BF16, addr_space="Shared",
                             name=f"ago{r}") for r in range(4)]
            # warm-up gather at t~0: absorbs the collective stack's cold
            # init + cross-core launch skew during quarter 0's compute
            warm_in = dram.tile([8, 512], BF16, name="warm_in")
            warm_out = [dram.tile([64, 512], BF16, addr_space="Shared",
                                  name=f"warm_out{i}") for i in range(4)]
            if not causal:
                qt_spill = dram.tile([DSH, S], BF16)  # Q^T rotated, [d, s]

            onesmat_sb = cns.tile([128, 128], BF16, tag="om")
            if causal:
                trimask_sb = cns.tile([128, 128], BF16, tag="trimask")
            fr_sb = cns.tile([128, S], BF16, tag="fr")
            fis_sb = cns.tile([128, S], BF16, tag="fis")

            kts = [akv.tile([128, S], BF16, tag=f"kth{h}", name=f"kth{h}")
                   for h in range(NH_LOC)]
            # all heads' V: [k-token part, token tile, 4 heads * 128 hd]
            vhs = akv.tile([128, QTILES, DSH], BF16, tag="vhs", name="vhs")

            attn_last_pe = {}
            pts_store = {}
            accs = {}
            vcps = []  # last V-evacuation copy per quarter (anchors)

            with (
                tc.tile_pool(name="xtp", bufs=2) as xtp,
                tc.tile_pool(name="qro", bufs=2) as qro,
                tc.tile_pool(name="qkw", bufs=3) as qkw,
                tc.tile_pool(name="qkd", bufs=2) as qkd,
                tc.tile_pool(name="vw", bufs=2) as vw,
                tc.tile_pool(name="apt", bufs=20) as apt,
                tc.tile_pool(name="awk", bufs=2) as awk,
                tc.tile_pool(name="aq", bufs=4) as aq,
            ):

                def emit_rope(tq, head, ps_in, is_q, qrot):
                    """RoPE one [128, 512] projected block. Half-swap via
                    two partition-moving DMAs; Q -> qrot SBUF (causal) or
                    DRAM spill (non-causal), K -> kts."""
                    sl = slice(tq * 512, (tq + 1) * 512)
                    qt_bf = qkd.tile([128, 512], BF16, tag="qt")
                    nc.scalar.copy(qt_bf[:], ps_in)
                    qsw = qkd.tile([128, 512], BF16, tag="qsw")
                    nc.scalar.dma_start(qsw[0:64, :], qt_bf[64:128, :])
                    nc.scalar.dma_start(qsw[64:128, :], qt_bf[0:64, :])
                    t1 = qkd.tile([128, 512], F32, tag="t1")
                    nc.vector.tensor_mul(t1[:], qt_bf[:], fr_sb[:, sl])
                    t2 = qkd.tile([128, 512], F32, tag="t2")
                    nc.vector.tensor_mul(t2[:], qsw[:], fis_sb[:, sl])
                    if is_q:
                        if causal:
                            nc.vector.tensor_add(qrot[:, head, :],
                                                 t1[:], t2[:])
                        else:
                            rot = qkd.tile([128, 512], BF16, tag="rot")
                            nc.vector.tensor_add(rot[:], t1[:], t2[:])
                            nc.sync.dma_start(
                                qt_spill[head * 128:(head + 1) * 128, sl],
                                rot[:])
                    else:
                        nc.vector.tensor_add(kts[head][:, sl], t1[:], t2[:])

                def emit_quarter(tq):
                    """Q/K/V projection of token quarter tq for the 4 local
                    heads. Returns the qrot tile (causal)."""
                    # x^T slice for this quarter, chunked; quarter 0 ramps
                    # finely so the first matmuls start ~3us in
                    xt_q = xtp.tile([128, KT, 512], BF16, tag="xt",
                                    name=f"xt{tq}")
                    if tq == 0:
                        chunks = [(0, 2), (2, 4), (4, 7), (7, 11),
                                  (11, 16), (16, 24), (24, 32)]
                    else:
                        chunks = [(0, 4), (4, 8), (8, 12), (12, 16),
                                  (16, 20), (20, 24), (24, 28), (28, 32)]
                    for (k0, k1) in chunks:
                        nc.sync.dma_start(
                            xt_q[:, k0:k1, :],
                            xT_d[k0 * 128:k1 * 128,
                                 tq * 512:(tq + 1) * 512]
                            .rearrange("(kt p) s -> p kt s", p=128),
                        )
                    if tq == 0:
                        nc.gpsimd.dma_start(fr_sb[:], fr_d[:, :])
                        nc.gpsimd.dma_start(fis_sb[:], fis_d[:, :])
                        nc.gpsimd.dma_start(onesmat_sb[:], onesmat_d[:, :])
                        if causal:
                            nc.gpsimd.dma_start(trimask_sb[:],
                                                trimask_d[:, :])
                        nc.gpsimd.dma_start(warm_in[:, :], fr_d[0:8, 0:512])
                        nc.gpsimd.collective_compute(
                            "AllGather",
                            mybir.AluOpType.bypass,
                            replica_groups=[list(range(N_CORES))],
                            ins=[warm_in[:].opt()],
                            outs=[warm_out[0][:].opt()],
                        )

                    qrot = None
                    if causal:
                        qrot = qro.tile([128, NH_LOC, 512], BF16,
                                        tag="qrot", name=f"qrot{tq}")
                    psQ = [ps.tile([128, 512], F32, tag=f"a{h}",
                                   name=f"qps{tq}_{h}", bufs=1)
                           for h in range(NH_LOC)]
                    # K psums live in the two 2-bank pair tiles (pA/pB) that
                    # the attention phase reuses for paired-exp score tiles
                    psKp = [ps.tile([128, 1024], F32, tag=f"p{i}",
                                    name=f"kps{tq}_{i}", bufs=1)
                            for i in range(2)]
                    psK = [psKp[h // 2][:, (h % 2) * 512:(h % 2 + 1) * 512]
                           for h in range(NH_LOC)]
                    wk_eng = nc.gpsimd if causal else nc.scalar
                    for wc in range(4):
                        for head in range(NH_LOC):
                            wq_c = qkw.tile([128, 8, 128], BF16, tag="wq")
                            nc.scalar.dma_start(
                                wq_c[:], wq_d[head, :, wc * 8:(wc + 1) * 8, :])
                            wk_c = qkw.tile([128, 8, 128], BF16, tag="wk")
                            wk_eng.dma_start(
                                wk_c[:], wk_d[head, :, wc * 8:(wc + 1) * 8, :])
                            for kt8 in range(8):
                                kt = wc * 8 + kt8
                                nc.tensor.matmul(
                                    psQ[head][:], wq_c[:, kt8, :],
                                    xt_q[:, kt, :],
                                    start=(wc == 0 and kt8 == 0),
                                    stop=(wc == 3 and kt8 == 7))
                                nc.tensor.matmul(
                                    psK[head], wk_c[:, kt8, :],
                                    xt_q[:, kt, :],
                                    start=(wc == 0 and kt8 == 0),
                                    stop=(wc == 3 and kt8 == 7))
                    for head in range(NH_LOC):
                        emit_rope(tq, head, psQ[head][:], True, qrot)
                        emit_rope(tq, head, psK[head], False, qrot)

                    # V projection for this quarter
                    psv = [ps.tile([128, 512], F32, tag=f"a{tt}",
                                   name=f"vps{tq}_{tt}", bufs=1)
                           for tt in range(4)]
                    for ktc in range(8):
                        wv_c = vw.tile([128, 4, 512], BF16, tag="wv")
                        wv_eng = nc.scalar if (ktc % 2 == 0 or not causal) \
                            else nc.gpsimd
                        wv_eng.dma_start(
                            wv_c[:], wv_d[:, ktc * 4:(ktc + 1) * 4, :])
                        for kt4 in range(4):
                            kt = ktc * 4 + kt4
                            for tt in range(4):
                                nc.tensor.matmul(
                                    psv[tt][:],
                                    xt_q[:, kt, tt * 128:(tt + 1) * 128],
                                    wv_c[:, kt4, :],
                                    start=(kt == 0), stop=(kt == KT - 1))
                    vcp = None
                    for tt in range(4):
                        vcp = nc.scalar.copy(vhs[:, tq * 4 + tt, :],
                                             psv[tt][:])
                    vcps.append(vcp)
                    return qrot

                def emit_scores(qr, head, qrot, zip_cb=None):
                    kt_h = kts[head]
                    if causal:
                        q_sb = qrot[:, head, :]
                    else:
                        q_tile = aq.tile([128, 512], BF16, tag="qsb",
                                         name=f"qsb{qr}_{head}")
                        nc.sync.dma_start(
                            q_tile[:],
                            qt_spill[head * 128:(head + 1) * 128,
                                     qr * 512:(qr + 1) * 512])
                        q_sb = q_tile[:]
                    nkt = (4 * qr + 4) if causal else QTILES
                    acc = awk.tile([128, 512], BF16, tag="acc",
                                   name=f"acc{qr}_{head}", bufs=4)
                    pts = []
                    # k-tiles processed in pairs sharing a 2-bank psum tile
                    # so ONE exp instruction covers both (the scalar ACT
                    # fixed cost paces late attention otherwise)
                    for pk in range(nkt // 2):
                        pair = ps.tile([128, 1024], F32, tag=f"p{pk % 2}",
                                       name=f"sp{qr}_{head}_{pk}", bufs=1)
                        pTp = apt.tile([128, 1024], BF16, tag="pT",
                                       name=f"pT{qr}_{head}_{pk}")
                        qlos = []
                        for half in range(2):
                            kt = 2 * pk + half
                            delta = kt - 4 * qr if causal else -1
                            qlo = max(0, delta) * 128
                            qlos.append(qlo)
                            nc.tensor.matmul(
                                pair[:, half * 512 + qlo:(half + 1) * 512],
                                kt_h[:, kt * 128:(kt + 1) * 128],
                                q_sb[:, qlo:],
                                start=True, stop=True)
                        if general:
                            for half in range(2):
                                kt = 2 * pk + half
                                hsl = slice(half * 512, (half + 1) * 512)
                                mt = awk.tile([128, 512], F32, tag="mt")
                                nc.sync.dma_start(
                                    mt[:],
                                    masktf_d[kt * 128:(kt + 1) * 128,
                                             qr * 512:(qr + 1) * 512])
                                msk = awk.tile([128, 512], F32, tag="msk")
                                nc.vector.scalar_tensor_tensor(
                                    msk[:], pair[:, hsl], SCALE, mt[:],
                                    op0=mybir.AluOpType.mult,
                                    op1=mybir.AluOpType.add)
                                nc.scalar.activation(
                                    pTp[:, hsl], msk[:],
                                    mybir.ActivationFunctionType.Exp)
                        else:
                            # one exp over both halves; the [qlo0:512+qlo1)
                            # sliver holds exp'd garbage that no consumer
                            # reads (PV and acc slice by qlo per half)
                            nc.scalar.activation(
                                pTp[:, qlos[0]:], pair[:, qlos[0]:],
                                mybir.ActivationFunctionType.Exp,
                                scale=SCALE)
                        for half in range(2):
                            kt = 2 * pk + half
                            delta = kt - 4 * qr if causal else -1
                            qlo = qlos[half]
                            base = half * 512
                            if delta >= 0:
                                # causal mask on the exp'd diagonal block
                                nc.vector.tensor_mul(
                                    pTp[:, base + qlo:base + qlo + 128],
                                    pTp[:, base + qlo:base + qlo + 128],
                                    trimask_sb[:])
                            if kt == 0:
                                nc.vector.tensor_copy(acc[:],
                                                      pTp[:, 0:512])
                            else:
                                nc.vector.tensor_add(
                                    acc[:, qlo:], acc[:, qlo:],
                                    pTp[:, base + qlo:base + 512])
                            pts.append((pTp, base, qlo))
                        if zip_cb is not None:
                            zip_cb(pk)
                    pts_store[(qr, head)] = pts
                    accs[(qr, head)] = acc

                def make_pv(qr, head):
                    """Returns (cb, finish): cb(pk) emits the two PV
                    matmuls for k-tiles 2pk/2pk+1 (zipped between the next
                    head's score pairs, filling the exp-paced PE bubbles);
                    finish() emits the denominator/normalize/agi tail."""
                    pts = pts_store.pop((qr, head))
                    acc = accs.pop((qr, head))
                    nkt = len(pts)
                    idx = (qr * 4 + head) % 2
                    ps_pv = ps.tile([128, 512], F32, tag=f"a{idx}",
                                    name=f"pv{qr}_{head}", bufs=1)
                    done = [0]

                    def cb(pk=None):
                        k0 = done[0]
                        k1 = nkt if pk is None else min(nkt, k0 + 2)
                        for kt in range(k0, k1):
                            pTp, base, qlo = pts[kt]
                            nc.tensor.matmul(
                                ps_pv[:, qlo:],
                                vhs[:, kt, head * 128:(head + 1) * 128],
                                pTp[:, base + qlo:base + 512],
                                start=(kt == 0), stop=(kt == nkt - 1))
                        done[0] = k1

                    def finish():
                        cb(None)
                        ps_rsb = ps.tile([128, 512], F32, tag=f"a{2 + idx}",
                                         name=f"rsb{qr}_{head}", bufs=1)
                        rsb_mm = nc.tensor.matmul(ps_rsb[:], onesmat_sb[:],
                                                  acc[:])
                        attn_last_pe[(qr, head)] = rsb_mm
                        rec_bc = awk.tile([128, 512], F32, tag="recb",
                                          bufs=2)
                        nc.vector.reciprocal_approx_fast(rec_bc[:],
                                                         ps_rsb[:])
                        at_sb = awk.tile([128, 512], BF16, tag="at", bufs=4)
                        nc.vector.tensor_mul(at_sb[:], ps_pv[:], rec_bc[:])
                        nc.gpsimd.dma_start(
                            agi[qr][head * 128:(head + 1) * 128, :],
                            at_sb[:])

                    return cb, finish

                def emit_ag(qr):
                    nc.gpsimd.collective_compute(
                        "AllGather",
                        mybir.AluOpType.bypass,
                        replica_groups=[list(range(N_CORES))],
                        ins=[agi[qr][:].opt()],
                        outs=[ago[qr][:].opt()],
                    )

                def emit_attention(r, qrot):
                    # software-pipelined heads: head h's exp-paced score
                    # pairs are zipped with head h-1's PV matmuls so the
                    # PE never waits on the scalar engine
                    for h in range(NH_LOC):
                        if h == 0:
                            emit_scores(r, h, qrot)
                        else:
                            cb, fin_prev = make_pv(r, h - 1)
                            emit_scores(r, h, qrot, zip_cb=cb)
                            fin_prev()
                    _, fin_last = make_pv(r, NH_LOC - 1)
                    fin_last()
                    emit_ag(r)

                def emit_warm_keepalive(i, anchor_inst):
                    """Non-causal only: keep the collective stack hot;
                    gpsimd FIFO blocking is acceptable there (agi writes
                    come long after)."""
                    wdma = nc.gpsimd.dma_start(warm_in[:, :],
                                               fr_d[0:8, 0:512])
                    tile.add_dep_helper(
                        wdma.ins, anchor_inst.ins, sync=True,
                        reason="pace keep-alive gather")
                    nc.gpsimd.collective_compute(
                        "AllGather",
                        mybir.AluOpType.bypass,
                        replica_groups=[list(range(N_CORES))],
                        ins=[warm_in[:].opt()],
                        outs=[warm_out[1 + i][:].opt()],
                    )

                for tq in range(QRANGES):
                    qrot = emit_quarter(tq)
                    if causal:
                        emit_attention(tq, qrot)
                    elif tq in (1, 2):
                        emit_warm_keepalive(tq - 1, vcps[-1])
                if not causal:
                    emit_warm_keepalive(2, vcps[-1])
                    for r in range(QRANGES):
                        emit_attention(r, None)

            # ---------- WO phase ----------
            with (
                tc.tile_pool(name="wop", bufs=1) as wop,
                tc.tile_pool(name="woa", bufs=2) as woa,
                tc.tile_pool(name="woo", bufs=2) as woo,
            ):
                # wo_sb rows are host-reordered h-major ([head][core][128])
                # matching the AllGather output layout
                wo_sb = wop.tile([128, KT, DSH], BF16, tag="wo")
                for ch in range(4):
                    nc.sync.dma_start(
                        wo_sb[:, ch * 8:(ch + 1) * 8, :],
                        wo_d[:, ch * 8:(ch + 1) * 8, :],
                    )

                def emit_wo(r, after=None, sync=True):
                    after_inst = attn_last_pe.get(after)
                    ps_os = [ps.tile([128, 512], F32, tag=f"a{qtl}",
                                     name=f"wops{r}_{qtl}", bufs=1)
                             for qtl in range(4)]
                    first_mm = [True]
                    for h in range(NH_LOC):
                        atqf = woa.tile([128, 8, 512], BF16, tag="atqf",
                                        name=f"atqf{r}_{h}")
                        src = (ago[r]
                               .rearrange("(c h p) q -> p c h q",
                                          c=8, h=4)[:, :, h, :])
                        nc.sync.dma_start(atqf[:], src)
                        for qtl in range(4):
                            for c in range(8):
                                gdt = h * 8 + c
                                mm = nc.tensor.matmul(
                                    ps_os[qtl][:],
                                    atqf[:, c, qtl * 128:(qtl + 1) * 128],
                                    wo_sb[:, gdt, :],
                                    start=(gdt == 0),
                                    stop=(gdt == KT - 1))
                                if first_mm[0] and after_inst is not None:
                                    tile.add_dep_helper(
                                        mm.ins, after_inst.ins,
                                        sync=sync,
                                        reason="order wo after attn")
                                    first_mm[0] = False
                    for qtl in range(4):
                        qt = r * 4 + qtl
                        o_sb = woo.tile([128, 512], F32, tag="osb",
                                        name=f"osb{qt}")
                        nc.vector.tensor_copy(o_sb[:], ps_os[qtl][:])
                        nc.sync.dma_start(
                            out_d[qt * 128:(qt + 1) * 128, :], o_sb[:])

                # strictly after ALL attention PE work: the last AllGather's
                # doorbell must ring before WO floods the PE queue, and the
                # gathers stay far ahead of WO's consumption anyway
                emit_wo(0, after=(3, 3), sync=True)
                emit_wo(1, after=(3, 3), sync=True)
                emit_wo(2, after=(3, 3), sync=True)
                emit_wo(3, after=(3, 3), sync=True)

    nc.compile()
    return nc


def _get_program(mode):
    if mode not in _PROGRAMS:
        _PROGRAMS[mode] = _build_program(mode)
    return _PROGRAMS[mode]


def _prep_inputs(x, wq, wk, wv, wo, freqs_real, freqs_imag, mask):
    """Host-side shard/layout prep. Returns (mode, in_maps)."""
    x = np.asarray(x, dtype=np.float32)
    wq = np.asarray(wq, dtype=np.float32)
    wk = np.asarray(wk, dtype=np.float32)
    wv = np.asarray(wv, dtype=np.float32)
    wo = np.asarray(wo, dtype=np.float32)
    fr = np.asarray(freqs_real, dtype=np.float32)
    fi = np.asarray(freqs_imag, dtype=np.float32)
    m = np.asarray(mask, dtype=np.float32).reshape(S, S)

    causal_ref = np.triu(np.full((S, S), np.float32(-1e9), dtype=np.float32), k=1)
    if np.array_equal(m, causal_ref):
        mode = "causal"
    elif not m.any():
        mode = "nomask"
    else:
        mode = "general"

    xT = np.ascontiguousarray(x.reshape(S, D).T)  # [D, S]
    xT_bf = xT.astype(ml_dtypes.bfloat16)

    # evens-first permutation of each head's 128 dims (for RoPE pair layout)
    idx = np.concatenate([np.arange(0, HD, 2), np.arange(1, HD, 2)])
    cols = np.concatenate([h * HD + idx for h in range(32)])
    wq_p = wq[:, cols]
    wk_p = wk[:, cols]

    # wo rows reordered h-major: [head h][core c][128 dims], matching the
    # AllGather output layout.
    row_order = np.concatenate(
        [np.arange(c * DSH + h * 128, c * DSH + (h + 1) * 128)
         for h in range(NH_LOC) for c in range(N_CORES)])
    wo_r = wo[row_order]

    fr128 = np.ascontiguousarray(np.concatenate([fr.T, fr.T], axis=0))   # [128, S]
    fis128 = np.ascontiguousarray(np.concatenate([-fi.T, fi.T], axis=0))

    onesmat = np.ones((128, 128), dtype=np.float32)

    in_maps = []
    for c in range(N_CORES):
        sl = slice(c * DSH, (c + 1) * DSH)

        def _wtile(a):
            # [D, C] -> [128p, KT, C] matching the SBUF tile layout
            return np.ascontiguousarray(
                a.reshape(KT, 128, a.shape[1]).transpose(1, 0, 2)
            ).astype(ml_dtypes.bfloat16)

        def _whead(a):
            # [D, 512] -> [NH_LOC, 128p, KT, HD]
            return np.ascontiguousarray(np.stack([
                _wtile(a[:, h * HD:(h + 1) * HD]) for h in range(NH_LOC)
            ]))

        im = {
            "xT": xT_bf,
            "wq": _whead(wq_p[:, sl]),
            "wk": _whead(wk_p[:, sl]),
            "wv": _wtile(wv[:, sl]),
            "wo": _wtile(wo_r[:, sl]),
            "fr128": fr128.astype(ml_dtypes.bfloat16),
            "fis128": fis128.astype(ml_dtypes.bfloat16),
            "onesmat": onesmat.astype(ml_dtypes.bfloat16),
        }
        if mode == "causal":
            # 0/1 mask tile in [k, q] layout: 1 iff k <= q
            trimask = (np.arange(128)[:, None] <= np.arange(128)[None, :])
            im["trimask"] = trimask.astype(ml_dtypes.bfloat16)
        if mode == "general":
            im["masktf"] = np.ascontiguousarray(m.T)
        in_maps.append(im)
    return mode, in_maps


def kernel(x, wq, wk, wv, wo, cache_k, cache_v, freqs_real, freqs_imag,
           mask, start_pos, **_unused):
    assert int(start_pos) == 0, "kernel hardcodes start_pos=0"
    mode, in_maps = _prep_inputs(x, wq, wk, wv, wo, freqs_real, freqs_imag, mask)
    nc = _get_program(mode)
    res = run_bass_kernel_spmd(nc, in_maps, core_ids=list(range(N_CORES)))
    out = np.concatenate([res.results[c]["out"] for c in range(N_CORES)], axis=1)
    return out.reshape(1, S, D).astype(np.float32)
